# revision 15
# baseline (speedup 1.0000x reference)
"""Bidirectional self-attention (B=4, T=2048, C=1024, H=16) on 8 TRN2 cores.

Sharding: core = b*2 + g  (b = batch 0..3, g = head-group 0..1, 8 heads each).
Each core computes, for its batch b and its 8 heads:
  QKV projection (its Wqkv column slice), full bidirectional attention, and
  the partial output projection (its Wo row slice).  The host sums the two
  partial outputs per batch and adds bo.

Per-core kernel layout (all on-chip, fp32 storage / fp32r matmuls):
  - x arrives pre-transposed as xT [C=1024, T=2048]  (c on partitions).
  - Q^T, K^T computed per head-pair as [128 (2 heads x 64 d), T] tiles.
  - V^T computed the same way, then PE-transposed into V_aug [128 t, 16 kb,
    130]: per head 64 V columns scaled by exp(mask_k) plus one exp(mask_k)
    column (softmax denominator comes out of the AV matmul for free).
  - scores^T per (pair, 512-q-chunk, 128-k-block): [128 k, 1024 (2 heads)]
    in PSUM, one exp per tile on ACT (bigger free dim amortizes ACT setup).
  - AV: y^T_aug [65, 512] accumulated over 16 k-blocks in PSUM; row 64 is
    the softmax denominator; normalize with partition-broadcast reciprocal.
  - Output projection contracts the 4 pair YT tiles against Wo rows,
    PSUM written straight back to DRAM.
No max-subtraction in softmax: scores*0.125 for these magnitudes are well
within exp range in fp32 (softmax is shift-invariant; reference matches).
"""

import os

import numpy as np

import concourse.bass as bass
import concourse.mybir as mybir
import concourse.tile as tile
from concourse.masks import make_identity
from concourse import library_config
from concourse.vector_clock import ScopedClock

B, T, C, H = 4, 2048, 1024, 16
D = C // H  # 64
NCORES = 8
NPAIR = 4  # head pairs per core (8 heads)
CC = C // 128  # 8 contraction chunks
TCH = T // 512  # 4 t-chunks of 512
KB = T // 128  # 16 k-blocks
QC = T // 512  # 4 q-chunks of 512

F32 = mybir.dt.float32
F32R = mybir.dt.float32r

_MM_DT = os.environ.get("KERNEL_MM_DTYPE", "fp32r")
# dtype for every tensor that feeds an fp32r matmul: the BIR verifier
# requires fp32r matmul operands to be *produced* as fp32r (rounded).
MDT = F32R if _MM_DT == "fp32r" else F32


def _mm(ap):
    return ap


class _SplitDrainTC(tile.TileContext):
    """TileContext whose tail drain carries at most one sync wait each.

    This toolchain's walrus lowers Drain to CTRL_NO_STRUCT which rejects
    more than one wait condition; the stock TileContext puts every
    outstanding proc's wait on a single Drain.
    """

    def _drain_and_barrier(self, tick_clock, wait_clock):
        drain_inst = self.nc.sync.drain()
        wait_clock.add_sem_waits(
            drain_inst.ins, ScopedClock({None: tick_clock.global_clock})
        )
        si = drain_inst.ins.sync_info
        waits = list(si.on_wait or []) if si is not None else []
        if len(waits) > 1:
            si.on_wait = waits[:1]
            for i in range(1, len(waits)):
                d = self.nc.sync.drain()
                d.ins.sync_info = mybir.SyncInfo(on_wait=waits[i : i + 1], on_update=[])
        self.nc.all_engine_barrier()
        assert self.sems is not None
        popped = self.nc._tile_sem_poison_stack.pop()
        assert popped is self._sem_poison
        self.nc.clear_and_free_semaphores(list(self.sems.allocated().values()))
        self.nc.all_engine_barrier()
        from waitsplit import split_excess_waits

        split_excess_waits(self.nc)


def build_program():
    nc = bass.Bass("TRN2", target_bir_lowering=False, debug=False, num_devices=NCORES)

    xT = nc.dram_tensor("xT", [C, T], MDT, kind="ExternalInput")
    wqkv = nc.dram_tensor("wqkv", [C, 3 * 512], MDT, kind="ExternalInput")
    bqkv_s = nc.dram_tensor("bqkv_s", [3 * 512], F32, kind="ExternalInput")
    mask = nc.dram_tensor("mask", [T], F32, kind="ExternalInput")
    wo_s = nc.dram_tensor("wo_s", [512, C], MDT, kind="ExternalInput")
    yp = nc.dram_tensor("yp", [T, C], F32, kind="ExternalOutput")

    Exp = mybir.ActivationFunctionType.Exp
    Mult = mybir.AluOpType.mult
    Add = mybir.AluOpType.add

    with _SplitDrainTC(nc) as tc:
        with (
            tc.tile_pool(name="const", bufs=1) as constp,
            tc.tile_pool(name="xt", bufs=1) as xtp,
            tc.tile_pool(name="w", bufs=3) as wp,
            tc.tile_pool(name="qkt", bufs=2) as qktp,
            tc.tile_pool(name="vt", bufs=2) as vtp,
            tc.tile_pool(name="vaug", bufs=2) as vaugp,
            tc.tile_pool(name="eab", bufs=3) as eabp,
            tc.tile_pool(name="yt", bufs=1) as ytp,
            tc.tile_pool(name="wo", bufs=1) as wop,
            tc.tile_pool(name="norm", bufs=2) as normp,
            tc.tile_pool(name="ps_small", bufs=2, space="PSUM") as ps_small,
            tc.tile_pool(name="ps_sc", bufs=2, space="PSUM") as ps_sc,
            tc.tile_pool(name="ps_y", bufs=2, space="PSUM") as ps_y,
        ):
            # ---- constants ----
            ident = constp.tile([128, 128], F32, tag="ident")
            make_identity(nc, ident[:])

            ones1 = constp.tile([1, 64], MDT, tag="ones1")
            ones1_f32 = constp.tile([1, 64], F32, tag="ones1f")
            nc.vector.memset(ones1_f32[:], 1.0)
            nc.vector.tensor_copy(ones1[:], ones1_f32[:])

            bias_sb = constp.tile([128, 12], F32, tag="bias")
            nc.sync.dma_start(
                out=bias_sb[:], in_=bqkv_s.ap().rearrange("(j p) -> p j", p=128)
            )

            mask_sb = constp.tile([128, KB], F32, tag="mask")
            nc.sync.dma_start(
                out=mask_sb[:], in_=mask.ap().rearrange("(blk p) -> p blk", p=128)
            )
            expmask = constp.tile([128, KB], F32, tag="expmask")
            nc.scalar.activation(expmask[:], mask_sb[:], Exp)

            # ---- x^T resident ----
            xts = []
            for cc in range(CC):
                t = xtp.tile([128, T], MDT, tag=f"xt{cc}", name=f"xt{cc}")
                nc.sync.dma_start(out=t[:], in_=xT[cc * 128 : (cc + 1) * 128, :])
                xts.append(t)

            yts = []
            for p in range(NPAIR):
                yts.append(ytp.tile([128, T], MDT, tag=f"yt{p}", name=f"yt{p}"))

            for p in range(NPAIR):
                # ---- projections for this pair ----
                qt = qktp.tile([128, T], MDT, tag="qt")
                kt = qktp.tile([128, T], MDT, tag="kt")
                vaug = vaugp.tile([128, KB, 130], MDT, tag="vaug")

                for ti, dest in ((0, qt), (1, kt), (2, None)):
                    j = ti * 4 + p
                    wt = wp.tile([128, CC, 128], MDT, tag="w")
                    nc.sync.dma_start(
                        out=wt[:],
                        in_=wqkv[:, j * 128 : (j + 1) * 128].rearrange(
                            "(cc p) d -> p cc d", p=128
                        ),
                    )
                    for tch in range(TCH):
                        ps = ps_small.tile([128, 512], F32, tag="sm")
                        for cc in range(CC):
                            nc.tensor.matmul(
                                ps[:],
                                _mm(wt[:, cc, :]),
                                _mm(xts[cc][:, tch * 512 : (tch + 1) * 512]),
                                start=(cc == 0),
                                stop=(cc == CC - 1),
                            )
                        if dest is not None:
                            # Q^T / K^T slice with per-partition bias add
                            nc.vector.tensor_scalar(
                                out=dest[:, tch * 512 : (tch + 1) * 512],
                                in0=ps[:],
                                scalar1=bias_sb[:, j : j + 1],
                                scalar2=None,
                                op0=Add,
                            )
                        else:
                            # V^T slice -> bias -> transpose into V_aug
                            vts = vtp.tile([128, 512], F32, tag="vts")
                            nc.vector.tensor_scalar(
                                out=vts[:],
                                in0=ps[:],
                                scalar1=bias_sb[:, j : j + 1],
                                scalar2=None,
                                op0=Add,
                            )
                            for blk in range(4):
                                tblk = tch * 4 + blk
                                pvt = ps_small.tile([128, 512], F32, tag="sm", name="pvt")
                                nc.tensor.transpose(
                                    pvt[:, 0:128],
                                    vts[:, blk * 128 : (blk + 1) * 128],
                                    ident[:],
                                )
                                # scale V rows by exp(mask_k); head A then B
                                nc.vector.tensor_scalar(
                                    out=vaug[:, tblk, 0:64],
                                    in0=pvt[:, 0:64],
                                    scalar1=expmask[:, tblk : tblk + 1],
                                    scalar2=None,
                                    op0=Mult,
                                )
                                nc.vector.tensor_scalar(
                                    out=vaug[:, tblk, 65:129],
                                    in0=pvt[:, 64:128],
                                    scalar1=expmask[:, tblk : tblk + 1],
                                    scalar2=None,
                                    op0=Mult,
                                )
                                nc.vector.tensor_copy(
                                    vaug[:, tblk, 64:65],
                                    expmask[:, tblk : tblk + 1],
                                )
                                nc.vector.tensor_copy(
                                    vaug[:, tblk, 129:130],
                                    expmask[:, tblk : tblk + 1],
                                )

                # ---- attention for this pair ----
                for qc in range(QC):
                    ya = ps_y.tile([65, 512], F32, tag="ya", bufs=1)
                    yb = ps_y.tile([65, 512], F32, tag="yb", bufs=1)
                    for kb in range(KB):
                        sc = ps_sc.tile([128, 1024], F32, tag="sc")
                        # scores^T, head A rows 0-63 / head B rows 64-127
                        nc.tensor.matmul(
                            sc[:, 0:512],
                            _mm(kt[0:64, kb * 128 : (kb + 1) * 128]),
                            _mm(qt[0:64, qc * 512 : (qc + 1) * 512]),
                        )
                        nc.tensor.matmul(
                            sc[:, 512:1024],
                            _mm(kt[64:128, kb * 128 : (kb + 1) * 128]),
                            _mm(qt[64:128, qc * 512 : (qc + 1) * 512]),
                        )
                        eab = eabp.tile([128, 1024], MDT, tag="eab")
                        nc.scalar.activation(eab[:], sc[:], Exp, scale=0.125)
                        nc.tensor.matmul(
                            ya[:],
                            _mm(vaug[:, kb, 0:65]),
                            _mm(eab[:, 0:512]),
                            start=(kb == 0),
                            stop=(kb == KB - 1),
                        )
                        nc.tensor.matmul(
                            yb[:],
                            _mm(vaug[:, kb, 65:130]),
                            _mm(eab[:, 512:1024]),
                            start=(kb == 0),
                            stop=(kb == KB - 1),
                        )
                    # normalize: rows/denominator -> YT
                    for hi, yh in ((0, ya), (1, yb)):
                        # one copy evicts Y rows + denom row, freeing the
                        # PSUM bank for the next q-chunk's AV accumulation
                        yu = normp.tile([65, 512], F32, tag="yu")
                        nc.vector.tensor_copy(yu[:], yh[:])
                        rr = normp.tile([1, 512], F32, tag="rr")
                        nc.vector.reciprocal(rr[:], yu[64:65, :])
                        rrr = normp.tile([1, 512], MDT, tag="rrr")
                        nc.vector.tensor_copy(rrr[:], rr[:])
                        # broadcast 1/denom across 64 partitions via K=1 matmul
                        pb = ps_small.tile([64, 512], F32, tag="sm", name="pb")
                        nc.tensor.matmul(pb[:], ones1[:], rrr[:])
                        nc.vector.tensor_tensor(
                            out=yts[p][
                                hi * 64 : hi * 64 + 64,
                                qc * 512 : (qc + 1) * 512,
                            ],
                            in0=yu[0:64, :],
                            in1=pb[:],
                            op=Mult,
                        )

            # ---- output projection ----
            wos = []
            for p in range(NPAIR):
                wot = wop.tile([128, C], MDT, tag=f"wo{p}", name=f"wo{p}")
                nc.sync.dma_start(out=wot[:], in_=wo_s[p * 128 : (p + 1) * 128, :])
                wos.append(wot)
            for tb in range(T // 128):
                for co in range(2):
                    po = ps_small.tile([128, 512], F32, tag="sm", name="po")
                    for p in range(NPAIR):
                        nc.tensor.matmul(
                            po[:],
                            _mm(yts[p][:, tb * 128 : (tb + 1) * 128]),
                            _mm(wos[p][:, co * 512 : (co + 1) * 512]),
                            start=(p == 0),
                            stop=(p == NPAIR - 1),
                        )
                    yo = normp.tile([128, 512], F32, tag="yo")
                    nc.scalar.copy(yo[:], po[:])
                    nc.sync.dma_start(
                        out=yp[tb * 128 : (tb + 1) * 128, co * 512 : (co + 1) * 512],
                        in_=yo[:],
                    )

    return nc


_PROGRAM = None
_RUN_KWARGS = {}  # test harness can set {"trace": True, "tmpdir": ...}
_LAST_RESULT = None


def _get_program():
    global _PROGRAM
    if _PROGRAM is None:
        _PROGRAM = build_program()
    return _PROGRAM


def kernel(x, attention_mask, Wqkv, bqkv, Wo, bo):
    from concourse.bass_utils import run_bass_kernel_spmd

    x = np.asarray(x, dtype=np.float32)
    attention_mask = np.asarray(attention_mask, dtype=np.float32)
    Wqkv = np.asarray(Wqkv, dtype=np.float32)
    bqkv = np.asarray(bqkv, dtype=np.float32)
    Wo = np.asarray(Wo, dtype=np.float32)
    bo = np.asarray(bo, dtype=np.float32)

    nc = _get_program()

    in_maps = []
    for core in range(NCORES):
        b, g = core // 2, core % 2
        cols = slice(g * 512, (g + 1) * 512)
        wq, wk, wv = Wqkv[:, 0:1024], Wqkv[:, 1024:2048], Wqkv[:, 2048:3072]
        in_maps.append(
            {
                "xT": np.ascontiguousarray(x[b].T),
                "wqkv": np.ascontiguousarray(
                    np.concatenate([wq[:, cols], wk[:, cols], wv[:, cols]], axis=1)
                ),
                "bqkv_s": np.ascontiguousarray(
                    np.concatenate(
                        [bqkv[0:1024][cols], bqkv[1024:2048][cols], bqkv[2048:3072][cols]]
                    )
                ),
                "mask": np.ascontiguousarray(attention_mask[b, 0, 0, :]),
                "wo_s": np.ascontiguousarray(Wo[g * 512 : (g + 1) * 512, :]),
            }
        )

    run_kwargs = dict(_RUN_KWARGS)
    res = run_bass_kernel_spmd(nc, in_maps, list(range(NCORES)), **run_kwargs)
    global _LAST_RESULT
    _LAST_RESULT = res
    out = np.empty((B, T, C), dtype=np.float32)
    for b in range(B):
        out[b] = res.results[2 * b]["yp"] + res.results[2 * b + 1]["yp"] + bo
    return out


# revision 17
# speedup vs baseline: 1.1160x; 1.1160x over previous
"""Bidirectional self-attention (B=4, T=2048, C=1024, H=16) on 8 TRN2 cores.

Sharding: core = b*2 + g  (b = batch 0..3, g = head-group 0..1, 8 heads each).
Each core computes, for its batch b and its 8 heads:
  QKV projection (its Wqkv column slice), full bidirectional attention, and
  the partial output projection (its Wo row slice).  The host sums the two
  partial outputs per batch and adds bo.

Per-core kernel layout (all on-chip, fp32 storage / fp32r matmuls):
  - x arrives pre-transposed as xT [C=1024, T=2048]  (c on partitions).
  - Q^T, K^T computed per head-pair as [128 (2 heads x 64 d), T] tiles.
  - V^T computed the same way, then PE-transposed into V_aug [128 t, 16 kb,
    130]: per head 64 V columns scaled by exp(mask_k) plus one exp(mask_k)
    column (softmax denominator comes out of the AV matmul for free).
  - scores^T per (pair, 512-q-chunk, 128-k-block): [128 k, 1024 (2 heads)]
    in PSUM, one exp per tile on ACT (bigger free dim amortizes ACT setup).
  - AV: y^T_aug [65, 512] accumulated over 16 k-blocks in PSUM; row 64 is
    the softmax denominator; normalize with partition-broadcast reciprocal.
  - Output projection contracts the 4 pair YT tiles against Wo rows,
    PSUM written straight back to DRAM.
No max-subtraction in softmax: scores*0.125 for these magnitudes are well
within exp range in fp32 (softmax is shift-invariant; reference matches).
"""

import os

import numpy as np

import concourse.bass as bass
import concourse.mybir as mybir
import concourse.tile as tile
from concourse.masks import make_identity
from concourse import library_config
from concourse.vector_clock import ScopedClock

B, T, C, H = 4, 2048, 1024, 16
D = C // H  # 64
NCORES = 8
NPAIR = 4  # head pairs per core (8 heads)
CC = C // 128  # 8 contraction chunks
TCH = T // 512  # 4 t-chunks of 512
KB = T // 128  # 16 k-blocks
QC = T // 512  # 4 q-chunks of 512

F32 = mybir.dt.float32
F32R = mybir.dt.float32r

_MM_DT = os.environ.get("KERNEL_MM_DTYPE", "fp32r")
# dtype for every tensor that feeds a matmul: the BIR verifier requires
# fp32r matmul operands to be *produced* as fp32r (rounded); bf16 operands
# are rounded by their producers naturally.
BF16 = mybir.dt.bfloat16
MDT = {"fp32r": F32R, "bf16": BF16, "fp32": F32}[_MM_DT]


def _mm(ap):
    return ap


class _SplitDrainTC(tile.TileContext):
    """TileContext whose tail drain carries at most one sync wait each.

    This toolchain's walrus lowers Drain to CTRL_NO_STRUCT which rejects
    more than one wait condition; the stock TileContext puts every
    outstanding proc's wait on a single Drain.
    """

    def _drain_and_barrier(self, tick_clock, wait_clock):
        drain_inst = self.nc.sync.drain()
        wait_clock.add_sem_waits(
            drain_inst.ins, ScopedClock({None: tick_clock.global_clock})
        )
        si = drain_inst.ins.sync_info
        waits = list(si.on_wait or []) if si is not None else []
        if len(waits) > 1:
            si.on_wait = waits[:1]
            for i in range(1, len(waits)):
                d = self.nc.sync.drain()
                d.ins.sync_info = mybir.SyncInfo(on_wait=waits[i : i + 1], on_update=[])
        self.nc.all_engine_barrier()
        assert self.sems is not None
        popped = self.nc._tile_sem_poison_stack.pop()
        assert popped is self._sem_poison
        self.nc.clear_and_free_semaphores(list(self.sems.allocated().values()))
        self.nc.all_engine_barrier()
        from waitsplit import split_excess_waits

        split_excess_waits(self.nc)


def build_program():
    nc = bass.Bass("TRN2", target_bir_lowering=False, debug=False, num_devices=NCORES)

    xT = nc.dram_tensor("xT", [C, T], MDT, kind="ExternalInput")
    wqkv = nc.dram_tensor("wqkv", [C, 3 * 512], MDT, kind="ExternalInput")
    bqkv_s = nc.dram_tensor("bqkv_s", [3 * 512], F32, kind="ExternalInput")
    mask = nc.dram_tensor("mask", [T], F32, kind="ExternalInput")
    wo_s = nc.dram_tensor("wo_s", [512, C], MDT, kind="ExternalInput")
    yp = nc.dram_tensor("yp", [T, C], F32, kind="ExternalOutput")

    Exp = mybir.ActivationFunctionType.Exp
    Mult = mybir.AluOpType.mult
    Add = mybir.AluOpType.add

    with _SplitDrainTC(nc) as tc:
        with (
            tc.tile_pool(name="const", bufs=1) as constp,
            tc.tile_pool(name="xt", bufs=1) as xtp,
            tc.tile_pool(name="w", bufs=3) as wp,
            tc.tile_pool(name="qkt", bufs=2) as qktp,
            tc.tile_pool(name="vt", bufs=2) as vtp,
            tc.tile_pool(name="vaug", bufs=2) as vaugp,
            tc.tile_pool(name="eab", bufs=3) as eabp,
            tc.tile_pool(name="yt", bufs=1) as ytp,
            tc.tile_pool(name="wo", bufs=1) as wop,
            tc.tile_pool(name="norm", bufs=2) as normp,
            tc.tile_pool(name="ps_small", bufs=2, space="PSUM") as ps_small,
            tc.tile_pool(name="ps_sc", bufs=2, space="PSUM") as ps_sc,
            tc.tile_pool(name="ps_y", bufs=2, space="PSUM") as ps_y,
        ):
            # ---- constants ----
            ident = constp.tile([128, 128], MDT, tag="ident")
            if MDT == F32:
                make_identity(nc, ident[:])
            else:
                ident_f32 = constp.tile([128, 128], F32, tag="identf")
                make_identity(nc, ident_f32[:])
                nc.vector.tensor_copy(ident[:], ident_f32[:])

            ones1 = constp.tile([1, 64], F32R, tag="ones1")
            ones1_f32 = constp.tile([1, 64], F32, tag="ones1f")
            nc.vector.memset(ones1_f32[:], 1.0)
            nc.vector.tensor_copy(ones1[:], ones1_f32[:])

            bias_sb = constp.tile([128, 12], F32, tag="bias")
            nc.sync.dma_start(
                out=bias_sb[:], in_=bqkv_s.ap().rearrange("(j p) -> p j", p=128)
            )

            mask_sb = constp.tile([128, KB], F32, tag="mask")
            nc.sync.dma_start(
                out=mask_sb[:], in_=mask.ap().rearrange("(blk p) -> p blk", p=128)
            )
            expmask = constp.tile([128, KB], F32, tag="expmask")
            nc.scalar.activation(expmask[:], mask_sb[:], Exp)

            # ---- x^T resident ----
            xts = []
            for cc in range(CC):
                t = xtp.tile([128, T], MDT, tag=f"xt{cc}", name=f"xt{cc}")
                nc.sync.dma_start(out=t[:], in_=xT[cc * 128 : (cc + 1) * 128, :])
                xts.append(t)

            yts = []
            for p in range(NPAIR):
                yts.append(ytp.tile([128, T], MDT, tag=f"yt{p}", name=f"yt{p}"))

            for p in range(NPAIR):
                # ---- projections for this pair ----
                qt = qktp.tile([128, T], MDT, tag="qt")
                kt = qktp.tile([128, T], MDT, tag="kt")
                vaug = vaugp.tile([128, KB, 130], MDT, tag="vaug")

                for ti, dest in ((0, qt), (1, kt), (2, None)):
                    j = ti * 4 + p
                    wt = wp.tile([128, CC, 128], MDT, tag="w")
                    nc.sync.dma_start(
                        out=wt[:],
                        in_=wqkv[:, j * 128 : (j + 1) * 128].rearrange(
                            "(cc p) d -> p cc d", p=128
                        ),
                    )
                    for tch in range(TCH):
                        ps = ps_small.tile([128, 512], F32, tag="sm")
                        for cc in range(CC):
                            nc.tensor.matmul(
                                ps[:],
                                _mm(wt[:, cc, :]),
                                _mm(xts[cc][:, tch * 512 : (tch + 1) * 512]),
                                start=(cc == 0),
                                stop=(cc == CC - 1),
                            )
                        if dest is not None:
                            # Q^T / K^T slice with per-partition bias add
                            nc.vector.tensor_scalar(
                                out=dest[:, tch * 512 : (tch + 1) * 512],
                                in0=ps[:],
                                scalar1=bias_sb[:, j : j + 1],
                                scalar2=None,
                                op0=Add,
                            )
                        else:
                            # V^T slice -> bias -> transpose into V_aug
                            vts = vtp.tile([128, 512], MDT, tag="vts")
                            nc.vector.tensor_scalar(
                                out=vts[:],
                                in0=ps[:],
                                scalar1=bias_sb[:, j : j + 1],
                                scalar2=None,
                                op0=Add,
                            )
                            for blk in range(4):
                                tblk = tch * 4 + blk
                                pvt = ps_small.tile([128, 512], MDT, tag="sm", name="pvt")
                                nc.tensor.transpose(
                                    pvt[:, 0:128],
                                    vts[:, blk * 128 : (blk + 1) * 128],
                                    ident[:],
                                )
                                # scale V rows by exp(mask_k); head A then B
                                nc.vector.tensor_scalar(
                                    out=vaug[:, tblk, 0:64],
                                    in0=pvt[:, 0:64],
                                    scalar1=expmask[:, tblk : tblk + 1],
                                    scalar2=None,
                                    op0=Mult,
                                )
                                nc.vector.tensor_scalar(
                                    out=vaug[:, tblk, 65:129],
                                    in0=pvt[:, 64:128],
                                    scalar1=expmask[:, tblk : tblk + 1],
                                    scalar2=None,
                                    op0=Mult,
                                )
                                nc.vector.tensor_copy(
                                    vaug[:, tblk, 64:65],
                                    expmask[:, tblk : tblk + 1],
                                )
                                nc.vector.tensor_copy(
                                    vaug[:, tblk, 129:130],
                                    expmask[:, tblk : tblk + 1],
                                )

                # ---- attention for this pair ----
                for qc in range(QC):
                    ya = ps_y.tile([65, 512], F32, tag="ya", bufs=1)
                    yb = ps_y.tile([65, 512], F32, tag="yb", bufs=1)
                    for kb in range(KB):
                        sc = ps_sc.tile([128, 1024], F32, tag="sc")
                        # scores^T, head A rows 0-63 / head B rows 64-127
                        nc.tensor.matmul(
                            sc[:, 0:512],
                            _mm(kt[0:64, kb * 128 : (kb + 1) * 128]),
                            _mm(qt[0:64, qc * 512 : (qc + 1) * 512]),
                        )
                        nc.tensor.matmul(
                            sc[:, 512:1024],
                            _mm(kt[64:128, kb * 128 : (kb + 1) * 128]),
                            _mm(qt[64:128, qc * 512 : (qc + 1) * 512]),
                        )
                        eab = eabp.tile([128, 1024], MDT, tag="eab")
                        nc.scalar.activation(eab[:], sc[:], Exp, scale=0.125)
                        nc.tensor.matmul(
                            ya[:],
                            _mm(vaug[:, kb, 0:65]),
                            _mm(eab[:, 0:512]),
                            start=(kb == 0),
                            stop=(kb == KB - 1),
                        )
                        nc.tensor.matmul(
                            yb[:],
                            _mm(vaug[:, kb, 65:130]),
                            _mm(eab[:, 512:1024]),
                            start=(kb == 0),
                            stop=(kb == KB - 1),
                        )
                    # normalize: rows/denominator -> YT
                    for hi, yh in ((0, ya), (1, yb)):
                        # one copy evicts Y rows + denom row, freeing the
                        # PSUM bank for the next q-chunk's AV accumulation
                        yu = normp.tile([65, 512], F32, tag="yu")
                        nc.vector.tensor_copy(yu[:], yh[:])
                        rr = normp.tile([1, 512], F32, tag="rr")
                        nc.vector.reciprocal(rr[:], yu[64:65, :])
                        rrr = normp.tile([1, 512], F32R, tag="rrr")
                        nc.vector.tensor_copy(rrr[:], rr[:])
                        # broadcast 1/denom across 64 partitions via K=1 matmul
                        pb = ps_small.tile([64, 512], F32, tag="sm", name="pb")
                        nc.tensor.matmul(pb[:], ones1[:], rrr[:])
                        nc.vector.tensor_tensor(
                            out=yts[p][
                                hi * 64 : hi * 64 + 64,
                                qc * 512 : (qc + 1) * 512,
                            ],
                            in0=yu[0:64, :],
                            in1=pb[:],
                            op=Mult,
                        )

            # ---- output projection ----
            wos = []
            for p in range(NPAIR):
                wot = wop.tile([128, C], MDT, tag=f"wo{p}", name=f"wo{p}")
                nc.sync.dma_start(out=wot[:], in_=wo_s[p * 128 : (p + 1) * 128, :])
                wos.append(wot)
            for tb in range(T // 128):
                for co in range(2):
                    po = ps_small.tile([128, 512], F32, tag="sm", name="po")
                    for p in range(NPAIR):
                        nc.tensor.matmul(
                            po[:],
                            _mm(yts[p][:, tb * 128 : (tb + 1) * 128]),
                            _mm(wos[p][:, co * 512 : (co + 1) * 512]),
                            start=(p == 0),
                            stop=(p == NPAIR - 1),
                        )
                    yo = normp.tile([128, 512], F32, tag="yo")
                    nc.scalar.copy(yo[:], po[:])
                    nc.sync.dma_start(
                        out=yp[tb * 128 : (tb + 1) * 128, co * 512 : (co + 1) * 512],
                        in_=yo[:],
                    )

    return nc


_PROGRAM = None
_RUN_KWARGS = {}  # test harness can set {"trace": True, "tmpdir": ...}
_LAST_RESULT = None


def _get_program():
    global _PROGRAM
    if _PROGRAM is None:
        _PROGRAM = build_program()
    return _PROGRAM


def kernel(x, attention_mask, Wqkv, bqkv, Wo, bo):
    from concourse.bass_utils import run_bass_kernel_spmd

    mdt_np = mybir.dt.np(MDT)

    x = np.asarray(x, dtype=np.float32)
    attention_mask = np.asarray(attention_mask, dtype=np.float32)
    Wqkv = np.asarray(Wqkv, dtype=np.float32)
    bqkv = np.asarray(bqkv, dtype=np.float32)
    Wo = np.asarray(Wo, dtype=np.float32)
    bo = np.asarray(bo, dtype=np.float32)

    nc = _get_program()

    in_maps = []
    for core in range(NCORES):
        b, g = core // 2, core % 2
        cols = slice(g * 512, (g + 1) * 512)
        wq, wk, wv = Wqkv[:, 0:1024], Wqkv[:, 1024:2048], Wqkv[:, 2048:3072]
        in_maps.append(
            {
                "xT": np.ascontiguousarray(x[b].T).astype(mdt_np),
                "wqkv": np.ascontiguousarray(
                    np.concatenate([wq[:, cols], wk[:, cols], wv[:, cols]], axis=1)
                ).astype(mdt_np),
                "bqkv_s": np.ascontiguousarray(
                    np.concatenate(
                        [bqkv[0:1024][cols], bqkv[1024:2048][cols], bqkv[2048:3072][cols]]
                    )
                ),
                "mask": np.ascontiguousarray(attention_mask[b, 0, 0, :]),
                "wo_s": np.ascontiguousarray(
                    Wo[g * 512 : (g + 1) * 512, :]
                ).astype(mdt_np),
            }
        )

    run_kwargs = dict(_RUN_KWARGS)
    res = run_bass_kernel_spmd(nc, in_maps, list(range(NCORES)), **run_kwargs)
    global _LAST_RESULT
    _LAST_RESULT = res
    out = np.empty((B, T, C), dtype=np.float32)
    for b in range(B):
        out[b] = res.results[2 * b]["yp"] + res.results[2 * b + 1]["yp"] + bo
    return out


# revision 18
# speedup vs baseline: 1.5587x; 1.3967x over previous
"""Bidirectional self-attention (B=4, T=2048, C=1024, H=16) on 8 TRN2 cores.

Sharding: core = b*2 + g  (b = batch 0..3, g = head-group 0..1, 8 heads each).
Each core computes, for its batch b and its 8 heads:
  QKV projection (its Wqkv column slice), full bidirectional attention, and
  the partial output projection (its Wo row slice).  The host sums the two
  partial outputs per batch and adds bo.

Per-core kernel layout (all on-chip, fp32 storage / fp32r matmuls):
  - x arrives pre-transposed as xT [C=1024, T=2048]  (c on partitions).
  - Q^T, K^T computed per head-pair as [128 (2 heads x 64 d), T] tiles.
  - V^T computed the same way, then PE-transposed into V_aug [128 t, 16 kb,
    130]: per head 64 V columns scaled by exp(mask_k) plus one exp(mask_k)
    column (softmax denominator comes out of the AV matmul for free).
  - scores^T per (pair, 512-q-chunk, 128-k-block): [128 k, 1024 (2 heads)]
    in PSUM, one exp per tile on ACT (bigger free dim amortizes ACT setup).
  - AV: y^T_aug [65, 512] accumulated over 16 k-blocks in PSUM; row 64 is
    the softmax denominator; normalize with partition-broadcast reciprocal.
  - Output projection contracts the 4 pair YT tiles against Wo rows,
    PSUM written straight back to DRAM.
No max-subtraction in softmax: scores*0.125 for these magnitudes are well
within exp range in fp32 (softmax is shift-invariant; reference matches).
"""

import os

import numpy as np

import concourse.bass as bass
import concourse.mybir as mybir
import concourse.tile as tile
from concourse.masks import make_identity
from concourse import library_config
from concourse.vector_clock import ScopedClock

B, T, C, H = 4, 2048, 1024, 16
D = C // H  # 64
NCORES = 8
NPAIR = 4  # head pairs per core (8 heads)
CC = C // 128  # 8 contraction chunks
TCH = T // 512  # 4 t-chunks of 512
KB = T // 128  # 16 k-blocks
QC = T // 512  # 4 q-chunks of 512

F32 = mybir.dt.float32
F32R = mybir.dt.float32r

_MM_DT = os.environ.get("KERNEL_MM_DTYPE", "fp32r")
# dtype for every tensor that feeds a matmul: the BIR verifier requires
# fp32r matmul operands to be *produced* as fp32r (rounded); bf16 operands
# are rounded by their producers naturally.
BF16 = mybir.dt.bfloat16
MDT = {"fp32r": F32R, "bf16": BF16, "fp32": F32}[_MM_DT]


def _mm(ap):
    return ap


class _SplitDrainTC(tile.TileContext):
    """TileContext whose tail drain carries at most one sync wait each.

    This toolchain's walrus lowers Drain to CTRL_NO_STRUCT which rejects
    more than one wait condition; the stock TileContext puts every
    outstanding proc's wait on a single Drain.
    """

    def _drain_and_barrier(self, tick_clock, wait_clock):
        drain_inst = self.nc.sync.drain()
        wait_clock.add_sem_waits(
            drain_inst.ins, ScopedClock({None: tick_clock.global_clock})
        )
        si = drain_inst.ins.sync_info
        waits = list(si.on_wait or []) if si is not None else []
        if len(waits) > 1:
            si.on_wait = waits[:1]
            for i in range(1, len(waits)):
                d = self.nc.sync.drain()
                d.ins.sync_info = mybir.SyncInfo(on_wait=waits[i : i + 1], on_update=[])
        self.nc.all_engine_barrier()
        assert self.sems is not None
        popped = self.nc._tile_sem_poison_stack.pop()
        assert popped is self._sem_poison
        self.nc.clear_and_free_semaphores(list(self.sems.allocated().values()))
        self.nc.all_engine_barrier()
        from waitsplit import split_excess_waits

        split_excess_waits(self.nc)


def build_program():
    nc = bass.Bass("TRN2", target_bir_lowering=False, debug=False, num_devices=NCORES)

    xT = nc.dram_tensor("xT", [C, T], MDT, kind="ExternalInput")
    wqkv = nc.dram_tensor("wqkv", [C, 3 * 512], MDT, kind="ExternalInput")
    bqkv_s = nc.dram_tensor("bqkv_s", [3 * 512], F32, kind="ExternalInput")
    mask = nc.dram_tensor("mask", [T], F32, kind="ExternalInput")
    wo_s = nc.dram_tensor("wo_s", [512, C], MDT, kind="ExternalInput")
    yp = nc.dram_tensor("yp", [T, C], F32, kind="ExternalOutput")

    Exp = mybir.ActivationFunctionType.Exp
    Mult = mybir.AluOpType.mult
    Add = mybir.AluOpType.add

    with _SplitDrainTC(nc) as tc:
        with (
            tc.tile_pool(name="const", bufs=1) as constp,
            tc.tile_pool(name="xt", bufs=1) as xtp,
            tc.tile_pool(name="w", bufs=3) as wp,
            tc.tile_pool(name="qkt", bufs=2) as qktp,
            tc.tile_pool(name="vt", bufs=2) as vtp,
            tc.tile_pool(name="vaug", bufs=2) as vaugp,
            tc.tile_pool(name="eab", bufs=3) as eabp,
            tc.tile_pool(name="yt", bufs=1) as ytp,
            tc.tile_pool(name="wo", bufs=1) as wop,
            tc.tile_pool(name="norm", bufs=2) as normp,
            tc.tile_pool(name="dram", bufs=4, space="DRAM") as dramp,
            tc.tile_pool(name="ps_small", bufs=2, space="PSUM") as ps_small,
            tc.tile_pool(name="ps_sc", bufs=2, space="PSUM") as ps_sc,
            tc.tile_pool(name="ps_y", bufs=2, space="PSUM") as ps_y,
        ):
            # ---- constants ----
            ident = constp.tile([128, 128], MDT, tag="ident")
            if MDT == F32:
                make_identity(nc, ident[:])
            else:
                ident_f32 = constp.tile([128, 128], F32, tag="identf")
                make_identity(nc, ident_f32[:])
                nc.vector.tensor_copy(ident[:], ident_f32[:])


            bias_sb = constp.tile([128, 12], F32, tag="bias")
            nc.sync.dma_start(
                out=bias_sb[:], in_=bqkv_s.ap().rearrange("(j p) -> p j", p=128)
            )

            mask_sb = constp.tile([128, KB], F32, tag="mask")
            nc.sync.dma_start(
                out=mask_sb[:], in_=mask.ap().rearrange("(blk p) -> p blk", p=128)
            )
            expmask = constp.tile([128, KB], F32, tag="expmask")
            nc.scalar.activation(expmask[:], mask_sb[:], Exp)

            # ---- x^T resident ----
            xts = []
            for cc in range(CC):
                t = xtp.tile([128, T], MDT, tag=f"xt{cc}", name=f"xt{cc}")
                nc.sync.dma_start(out=t[:], in_=xT[cc * 128 : (cc + 1) * 128, :])
                xts.append(t)

            yts = []
            for p in range(NPAIR):
                yts.append(ytp.tile([128, T], MDT, tag=f"yt{p}", name=f"yt{p}"))

            for p in range(NPAIR):
                # ---- projections for this pair ----
                qt = qktp.tile([128, T], MDT, tag="qt")
                kt = qktp.tile([128, T], MDT, tag="kt")
                vaug = vaugp.tile([128, KB, 130], MDT, tag="vaug")

                for ti, dest in ((0, qt), (1, kt), (2, None)):
                    j = ti * 4 + p
                    wt = wp.tile([128, CC, 128], MDT, tag="w")
                    nc.sync.dma_start(
                        out=wt[:],
                        in_=wqkv[:, j * 128 : (j + 1) * 128].rearrange(
                            "(cc p) d -> p cc d", p=128
                        ),
                    )
                    for tch in range(TCH):
                        ps = ps_small.tile([128, 512], F32, tag="sm")
                        for cc in range(CC):
                            nc.tensor.matmul(
                                ps[:],
                                _mm(wt[:, cc, :]),
                                _mm(xts[cc][:, tch * 512 : (tch + 1) * 512]),
                                start=(cc == 0),
                                stop=(cc == CC - 1),
                            )
                        if dest is not None:
                            # Q^T / K^T slice with per-partition bias add
                            nc.vector.tensor_scalar(
                                out=dest[:, tch * 512 : (tch + 1) * 512],
                                in0=ps[:],
                                scalar1=bias_sb[:, j : j + 1],
                                scalar2=None,
                                op0=Add,
                            )
                        else:
                            # V^T slice -> bias -> transpose into V_aug
                            vts = vtp.tile([128, 512], MDT, tag="vts")
                            nc.vector.tensor_scalar(
                                out=vts[:],
                                in0=ps[:],
                                scalar1=bias_sb[:, j : j + 1],
                                scalar2=None,
                                op0=Add,
                            )
                            for blk in range(4):
                                tblk = tch * 4 + blk
                                pvt = ps_small.tile([128, 512], MDT, tag="sm", name="pvt")
                                nc.tensor.transpose(
                                    pvt[:, 0:128],
                                    vts[:, blk * 128 : (blk + 1) * 128],
                                    ident[:],
                                )
                                # scale V rows by exp(mask_k); head A then B
                                nc.vector.tensor_scalar(
                                    out=vaug[:, tblk, 0:64],
                                    in0=pvt[:, 0:64],
                                    scalar1=expmask[:, tblk : tblk + 1],
                                    scalar2=None,
                                    op0=Mult,
                                )
                                nc.vector.tensor_scalar(
                                    out=vaug[:, tblk, 65:129],
                                    in0=pvt[:, 64:128],
                                    scalar1=expmask[:, tblk : tblk + 1],
                                    scalar2=None,
                                    op0=Mult,
                                )
                                nc.vector.tensor_copy(
                                    vaug[:, tblk, 64:65],
                                    expmask[:, tblk : tblk + 1],
                                )
                                nc.vector.tensor_copy(
                                    vaug[:, tblk, 129:130],
                                    expmask[:, tblk : tblk + 1],
                                )

                # ---- attention for this pair ----
                for qc in range(QC):
                    ya = ps_y.tile([65, 512], F32, tag="ya", bufs=1)
                    yb = ps_y.tile([65, 512], F32, tag="yb", bufs=1)
                    for kb in range(KB):
                        sc = ps_sc.tile([128, 1024], F32, tag="sc")
                        # scores^T, head A rows 0-63 / head B rows 64-127
                        nc.tensor.matmul(
                            sc[:, 0:512],
                            _mm(kt[0:64, kb * 128 : (kb + 1) * 128]),
                            _mm(qt[0:64, qc * 512 : (qc + 1) * 512]),
                        )
                        nc.tensor.matmul(
                            sc[:, 512:1024],
                            _mm(kt[64:128, kb * 128 : (kb + 1) * 128]),
                            _mm(qt[64:128, qc * 512 : (qc + 1) * 512]),
                        )
                        eab = eabp.tile([128, 1024], MDT, tag="eab")
                        nc.scalar.activation(eab[:], sc[:], Exp, scale=0.125)
                        nc.tensor.matmul(
                            ya[:],
                            _mm(vaug[:, kb, 0:65]),
                            _mm(eab[:, 0:512]),
                            start=(kb == 0),
                            stop=(kb == KB - 1),
                        )
                        nc.tensor.matmul(
                            yb[:],
                            _mm(vaug[:, kb, 65:130]),
                            _mm(eab[:, 512:1024]),
                            start=(kb == 0),
                            stop=(kb == KB - 1),
                        )
                    # normalize: rows/denominator -> YT
                    for hi, yh in ((0, ya), (1, yb)):
                        # one copy evicts Y rows + denom row, freeing the
                        # PSUM bank for the next q-chunk's AV accumulation
                        yu = normp.tile([65, 512], F32, tag="yu")
                        nc.vector.tensor_copy(yu[:], yh[:])
                        rr = normp.tile([1, 512], F32, tag="rr")
                        nc.vector.reciprocal(rr[:], yu[64:65, :])
                        # broadcast 1/denom across 64 partitions with a DRAM
                        # round-trip (keeps the in-order PE queue out of the
                        # slow reciprocal's dependency chain)
                        scd = dramp.tile([1, 512], F32, tag="dscr")
                        nc.sync.dma_start(out=scd[:], in_=rr[:])
                        dbc = normp.tile([64, 512], F32, tag="dbc")
                        nc.sync.dma_start(
                            out=dbc[:], in_=scd[:].to_broadcast((64, 512))
                        )
                        nc.vector.tensor_tensor(
                            out=yts[p][
                                hi * 64 : hi * 64 + 64,
                                qc * 512 : (qc + 1) * 512,
                            ],
                            in0=yu[0:64, :],
                            in1=dbc[:],
                            op=Mult,
                        )

            # ---- output projection ----
            wos = []
            for p in range(NPAIR):
                wot = wop.tile([128, C], MDT, tag=f"wo{p}", name=f"wo{p}")
                nc.sync.dma_start(out=wot[:], in_=wo_s[p * 128 : (p + 1) * 128, :])
                wos.append(wot)
            for tb in range(T // 128):
                for co in range(2):
                    po = ps_small.tile([128, 512], F32, tag="sm", name="po")
                    for p in range(NPAIR):
                        nc.tensor.matmul(
                            po[:],
                            _mm(yts[p][:, tb * 128 : (tb + 1) * 128]),
                            _mm(wos[p][:, co * 512 : (co + 1) * 512]),
                            start=(p == 0),
                            stop=(p == NPAIR - 1),
                        )
                    yo = normp.tile([128, 512], F32, tag="yo")
                    nc.scalar.copy(yo[:], po[:])
                    nc.sync.dma_start(
                        out=yp[tb * 128 : (tb + 1) * 128, co * 512 : (co + 1) * 512],
                        in_=yo[:],
                    )

    return nc


_PROGRAM = None
_RUN_KWARGS = {}  # test harness can set {"trace": True, "tmpdir": ...}
_LAST_RESULT = None


def _get_program():
    global _PROGRAM
    if _PROGRAM is None:
        _PROGRAM = build_program()
    return _PROGRAM


def kernel(x, attention_mask, Wqkv, bqkv, Wo, bo):
    from concourse.bass_utils import run_bass_kernel_spmd

    mdt_np = mybir.dt.np(MDT)

    x = np.asarray(x, dtype=np.float32)
    attention_mask = np.asarray(attention_mask, dtype=np.float32)
    Wqkv = np.asarray(Wqkv, dtype=np.float32)
    bqkv = np.asarray(bqkv, dtype=np.float32)
    Wo = np.asarray(Wo, dtype=np.float32)
    bo = np.asarray(bo, dtype=np.float32)

    nc = _get_program()

    in_maps = []
    for core in range(NCORES):
        b, g = core // 2, core % 2
        cols = slice(g * 512, (g + 1) * 512)
        wq, wk, wv = Wqkv[:, 0:1024], Wqkv[:, 1024:2048], Wqkv[:, 2048:3072]
        in_maps.append(
            {
                "xT": np.ascontiguousarray(x[b].T).astype(mdt_np),
                "wqkv": np.ascontiguousarray(
                    np.concatenate([wq[:, cols], wk[:, cols], wv[:, cols]], axis=1)
                ).astype(mdt_np),
                "bqkv_s": np.ascontiguousarray(
                    np.concatenate(
                        [bqkv[0:1024][cols], bqkv[1024:2048][cols], bqkv[2048:3072][cols]]
                    )
                ),
                "mask": np.ascontiguousarray(attention_mask[b, 0, 0, :]),
                "wo_s": np.ascontiguousarray(
                    Wo[g * 512 : (g + 1) * 512, :]
                ).astype(mdt_np),
            }
        )

    run_kwargs = dict(_RUN_KWARGS)
    res = run_bass_kernel_spmd(nc, in_maps, list(range(NCORES)), **run_kwargs)
    global _LAST_RESULT
    _LAST_RESULT = res
    out = np.empty((B, T, C), dtype=np.float32)
    for b in range(B):
        out[b] = res.results[2 * b]["yp"] + res.results[2 * b + 1]["yp"] + bo
    return out
